# revision 2
# baseline (speedup 1.0000x reference)
"""GCN sampling kernel v5: chunked int16 dma_gather + PE selector pooling.

Replaces v4's per-128-row indirect DMAs (994ns SWDGE fixed cost each, 380
instructions -> 394us Pool wall) with ~91 large dma_gather instructions
(3000-5000 descriptors each). Edges are grouped by (slot-block, feature-row
chunk); each (range, chunk) gather lands edges at 128-aligned position
blocks; a DVE-built one-hot selector matmul pools positions -> slots in
PSUM, accumulated across the 13 chunks. The W1 stage (transpose + matmul +
ReLU) and layer-1 SEL matmul are v4's, software-pipelined one range behind
the pooling.
"""

import sys

sys.path.insert(0, "/opt/trn_rl_repo")

from contextlib import ExitStack

import numpy as np
import ml_dtypes

N0, N1, N2 = 409600, 40960, 4096
F = 10
IN_F, HID, NCLS = 512, 256, 64
NC_N = 8
DST_PC = N2 // NC_N         # 512 dst nodes per core
BLK = 128
CH = 32767                  # int16-addressable feature-row window
NCH = -(-N0 // CH)          # 13
RNG = 5                     # slot-blocks per range (PSUM budget)

_BUILT = {}


def _legalize_waits(bir: bytes) -> bytes:
    import orjson

    j = orjson.loads(bir)
    n_new = 0
    for fn in j["functions"]:
        for bb in fn["blocks"]:
            insts = bb["instructions"]
            out = []
            for inst in insts:
                si = inst.get("sync_info")
                waits = si.get("on_wait") if si else None
                if waits and len(waits) > 1:
                    for w in waits[:-1]:
                        n_new += 1
                        out.append({
                            "debug": inst.get("debug", 0),
                            "engine": inst["engine"],
                            "ins": [],
                            "name": f"{inst['name']}_esw{n_new}",
                            "opcode": "EventSemaphore",
                            "outs": [],
                            "sync_info": {"on_update": [], "on_wait": [w]},
                        })
                    si["on_wait"] = [waits[-1]]
                out.append(inst)
            bb["instructions"] = out
    return orjson.dumps(j)


def _install_patch():
    import concourse.bass as bass

    if getattr(bass.Bass, "_gcn_wait_patch", False):
        return
    orig = bass.Bass.to_json_bytes

    def to_json_bytes(self, *a, **kw):
        return _legalize_waits(orig(self, *a, **kw))

    bass.Bass.to_json_bytes = to_json_bytes
    bass.Bass._gcn_wait_patch = True


def _ranges(nb1):
    out = []
    st = 0
    while st < nb1:
        out.append(list(range(st, min(st + RNG, nb1))))
        st += RNG
    return out


def build_nc(nb1, npb_key):
    """npb_key: flattened tuple of npb[sb][c] (position blocks per
    (slot-block, chunk)), shared by all cores."""
    _install_patch()
    import concourse.bacc as bacc
    import concourse.tile as tile
    from concourse import mybir

    f32 = mybir.dt.float32
    bf16 = mybir.dt.bfloat16
    i16 = mybir.dt.int16

    npb = np.asarray(npb_key, np.int64).reshape(nb1, NCH)
    ranges = _ranges(nb1)
    # emission-ordered pb bookkeeping
    # per (r, c): list of (sb, piece, local_off_blocks), npos
    rc_pbs = {}
    pb_col = {}          # (sb, c, piece) -> slotw column
    idx_off = {}         # (r, c) -> 16-col offset into idx table
    ncol = 0
    tot16 = 0
    maxnpos = 0
    for ri, r in enumerate(ranges):
        for c in range(NCH):
            lst = []
            off = 0
            for sb in r:
                for piece in range(npb[sb][c]):
                    lst.append((sb, piece, off))
                    pb_col[(sb, c, piece)] = ncol
                    ncol += 1
                    off += 1
            rc_pbs[(ri, c)] = lst
            idx_off[(ri, c)] = tot16
            npos = off * 128
            tot16 += npos // 16
            maxnpos = max(maxnpos, npos)

    nc = bacc.Bacc("TRN2", target_bir_lowering=False, debug=False,
                   num_devices=NC_N, num_swdge_queues=4)

    feat = nc.dram_tensor("feat", [N0, IN_F], bf16, kind="ExternalInput")
    w1 = nc.dram_tensor("w1", [128, 4 * HID], bf16, kind="ExternalInput")
    w2 = nc.dram_tensor("w2", [128, 2 * NCLS], bf16, kind="ExternalInput")
    ident_in = nc.dram_tensor("ident", [128, 128], bf16,
                              kind="ExternalInput")
    iota_in = nc.dram_tensor("iota", [128, 128], bf16, kind="ExternalInput")
    gidx = nc.dram_tensor("gidx", [128, tot16], i16, kind="ExternalInput")
    slw = nc.dram_tensor("slw", [128, ncol], bf16, kind="ExternalInput")
    sel1 = nc.dram_tensor("sel1", [128, nb1 * 4 * 128], mybir.dt.float8e4,
                          kind="ExternalInput")
    outT = nc.dram_tensor("outT", [NCLS, DST_PC], f32, kind="ExternalOutput")

    with tile.TileContext(nc) as tc, ExitStack() as ctx:
        consts = ctx.enter_context(tc.tile_pool(name="consts", bufs=1))
        gpool = ctx.enter_context(tc.tile_pool(name="gbuf", bufs=8))
        selp = ctx.enter_context(tc.tile_pool(name="selp", bufs=6))
        p0pool = ctx.enter_context(tc.tile_pool(name="p0", bufs=6))
        hpool = ctx.enter_context(tc.tile_pool(name="hc", bufs=4))
        h1pool = ctx.enter_context(tc.tile_pool(name="h1", bufs=1))
        p2pool = ctx.enter_context(tc.tile_pool(name="p2sb", bufs=1))
        tpool = ctx.enter_context(tc.tile_pool(name="temps", bufs=2))
        ps_bank = ctx.enter_context(tc.tile_pool(name="ps_bank", bufs=1,
                                                 space="PSUM"))
        ps_tr = ctx.enter_context(tc.tile_pool(name="ps_tr", bufs=2,
                                               space="PSUM"))
        ps_w1 = ctx.enter_context(tc.tile_pool(name="ps_w1", bufs=1,
                                               space="PSUM"))

        idx_t = consts.tile([128, tot16], i16, name="idx_t")
        nc.sync.dma_start(idx_t[:], gidx.ap())
        slw_t = consts.tile([128, ncol], bf16, name="slw_t")
        nc.sync.dma_start(slw_t[:], slw.ap())
        iota_t = consts.tile([128, 128], bf16, name="iota_t")
        nc.sync.dma_start(iota_t[:], iota_in.ap())
        ident = consts.tile([128, 128], bf16, name="ident_t")
        nc.sync.dma_start(ident[:], ident_in.ap())
        w1t = consts.tile([128, 4 * HID], bf16, name="w1t")
        nc.sync.dma_start(w1t[:], w1.ap())
        w2t = consts.tile([128, 2 * NCLS], bf16, name="w2t")
        nc.sync.dma_start(w2t[:], w2.ap())
        sel1_t = consts.tile([128, nb1 * 4 * 128], mybir.dt.float8e4,
                             name="sel1_t")

        h1t = h1pool.tile([128, nb1 * HID], bf16, name="h1t")
        pooled2 = p2pool.tile([128, 4 * HID], f32, name="pooled2")

        banks = {}

        def emit_pooling(ri, c, drain):
            pbs = rc_pbs[(ri, c)]
            npos = len(pbs) * 128
            buf = gpool.tile([128, (maxnpos // 128) * IN_F], bf16,
                             tag="gb", name=f"gb_{ri}_{c}")
            o16 = idx_off[(ri, c)]
            nc.gpsimd.dma_gather(
                out_ap=buf[:, :len(pbs) * IN_F].rearrange(
                    "p (b d) -> p b d", b=len(pbs)),
                in_ap=feat.ap()[c * CH:min((c + 1) * CH, N0)],
                idxs_ap=idx_t[:, o16:o16 + npos // 16],
                num_idxs=npos, num_idxs_reg=npos, elem_size=IN_F,
            )
            npbs = len(pbs)
            col0 = pb_col[(pbs[0][0], c, 0)]
            sel = selp.tile([128, maxnpos], bf16, tag="sel",
                            name=f"sel_{ri}_{c}")
            nc.vector.tensor_tensor(
                out=sel[:, :npbs * 128].rearrange("p (n d) -> p n d", n=npbs),
                in0=iota_t[:].rearrange("p (u d) -> p u d", u=1).broadcast_to(
                    [128, npbs, 128]),
                in1=slw_t[:, col0:col0 + npbs].rearrange(
                    "p (n u) -> p n u", u=1).broadcast_to([128, npbs, 128]),
                op=mybir.AluOpType.is_equal)
            for sb, piece, off in pbs:
                if c == 0 and piece == 0:
                    banks[sb] = ps_bank.tile(
                        [128, IN_F], f32, tag=f"bk{sb % RNG}", space="PSUM",
                        name=f"bank_{sb}")
                nc.tensor.matmul(
                    banks[sb][:], lhsT=sel[:, off * 128:(off + 1) * 128],
                    rhs=buf[:, off * IN_F:(off + 1) * IN_F],
                    start=(c == 0 and piece == 0),
                    stop=(c == NCH - 1 and piece == npb[sb][c] - 1),
                    skip_group_check=True)
                drain(1 if len(backlog) < 30 else 2)

        p0tiles = {}

        def evict_piece(b):
            # evict bank b -> pooled0 block, freeing the PSUM bank
            # (alternate DVE/Act so the 5 evicts at a range boundary overlap)
            p0 = p0pool.tile([128, IN_F], bf16, tag="p0", name=f"p0_{b}")
            nc.scalar.activation(p0[:], banks[b][:],
                                 mybir.ActivationFunctionType.Copy)
            p0tiles[b] = p0

        def back_ops(ri):
            """Yield fine-grained closures for the range's W1 stage + SEL1,
            to be interleaved between pooling matmuls so PE never blocks on
            a psum->sbuf copy round-trip."""
            r = ranges[ri]
            first_range = (ri == 0)
            state = {}

            def mk_tr(b, fc):
                def op():
                    p0 = p0tiles[b]
                    ptr = ps_tr.tile([128, 128], f32, tag="ptr", space="PSUM",
                                     name=f"ptr_{b}_{fc}")
                    nc.tensor.matmul(ptr[:],
                                     lhsT=p0[:, fc * 128:(fc + 1) * 128],
                                     rhs=ident[:], start=True, stop=True,
                                     skip_group_check=True)
                    hc_sb = hpool.tile([128, 128], bf16, tag=f"hc{fc % 2}",
                                       name=f"hc_{b}_{fc}")
                    if fc % 2 == 0:
                        nc.vector.tensor_copy(hc_sb[:], ptr[:])
                    else:
                        nc.scalar.activation(
                            hc_sb[:], ptr[:],
                            mybir.ActivationFunctionType.Copy)
                    state[(b, fc)] = hc_sb
                return op

            def mk_mm(b, fc):
                def op():
                    if fc == 0:
                        state[b] = ps_w1.tile([128, HID], f32, tag="pw",
                                              space="PSUM", name=f"pw_{b}")
                    nc.tensor.matmul(state[b][:], lhsT=state.pop((b, fc))[:],
                                     rhs=w1t[:, fc * HID:(fc + 1) * HID],
                                     start=(fc == 0), stop=(fc == 3),
                                     skip_group_check=True)
                return op

            def mk_relu(b):
                def op():
                    nc.scalar.activation(h1t[:, b * HID:(b + 1) * HID],
                                         state.pop(b)[:],
                                         mybir.ActivationFunctionType.Relu)
                    p0tiles.pop(b)
                return op

            def mk_sel1(rb, j, b):
                def op():
                    if j == 0:
                        state[("pl", rb)] = ps_w1.tile(
                            [128, HID], f32, tag="pw", space="PSUM",
                            name=f"pl_{ri}_{rb}")
                    nc.tensor.matmul(
                        state[("pl", rb)][:],
                        lhsT=sel1_t[:, (b * 4 + rb) * 128:
                                    (b * 4 + rb) * 128 + 128],
                        rhs=h1t[:, b * HID:(b + 1) * HID],
                        start=(j == 0), stop=(j == len(r) - 1),
                        skip_group_check=True)
                return op

            def mk_add(rb):
                def op():
                    pl = state.pop(("pl", rb))
                    if first_range:
                        nc.vector.tensor_copy(
                            pooled2[:, rb * HID:(rb + 1) * HID], pl[:])
                    else:
                        nc.vector.tensor_tensor(
                            out=pooled2[:, rb * HID:(rb + 1) * HID],
                            in0=pooled2[:, rb * HID:(rb + 1) * HID],
                            in1=pl[:], op=mybir.AluOpType.add)
                return op

            def mk_w2(rb, hc, stage):
                # stage 0: copy pooled2->bf16 (+first transpose)
                # stage 1: transpose chunk hc; stage 2: W2 matmul hc
                # stage 3: copy out + DMA
                def op():
                    if stage == 0:
                        p2 = tpool.tile([128, HID], bf16, tag="p2",
                                        name=f"p2_{rb}")
                        nc.vector.tensor_copy(
                            p2[:], pooled2[:, rb * HID:(rb + 1) * HID])
                        state[("p2", rb)] = p2
                        state[("p2T", rb)] = tpool.tile(
                            [128, 2 * 128], bf16, tag="p2T", name=f"p2T_{rb}")
                    elif stage == 1:
                        p2 = state[("p2", rb)]
                        ptr = ps_tr.tile([128, 128], f32, tag="ptr",
                                         space="PSUM",
                                         name=f"ptrT_{rb}_{hc}")
                        nc.tensor.matmul(
                            ptr[:], lhsT=p2[:, hc * 128:(hc + 1) * 128],
                            rhs=ident[:], start=True, stop=True,
                            skip_group_check=True)
                        nc.vector.tensor_copy(
                            state[("p2T", rb)][:, hc * 128:(hc + 1) * 128],
                            ptr[:])
                    elif stage == 2:
                        if hc == 0:
                            state[("po", rb)] = ps_w1.tile(
                                [128, HID], f32, tag="pw", space="PSUM",
                                name=f"po_{rb}")
                        nc.tensor.matmul(
                            state[("po", rb)][0:NCLS, 0:128],
                            lhsT=w2t[:, hc * NCLS:(hc + 1) * NCLS],
                            rhs=state[("p2T", rb)][:, hc * 128:(hc + 1) * 128],
                            start=(hc == 0), stop=(hc == 1),
                            skip_group_check=True)
                    else:
                        osb = tpool.tile([NCLS, 128], f32, tag="osb",
                                         name=f"osb_{rb}")
                        nc.vector.tensor_copy(osb[:],
                                              state.pop(("po", rb))[0:NCLS,
                                                                    0:128])
                        nc.sync.dma_start(
                            outT.ap()[:, rb * 128:(rb + 1) * 128], osb[:])
                        state.pop(("p2", rb))
                        state.pop(("p2T", rb))
                return op

            last_range = (ri == len(ranges) - 1)
            if not last_range:
                for b in r:
                    for fc in range(4):
                        yield mk_tr(b, fc)
                    for fc in range(4):
                        yield mk_mm(b, fc)
                    yield mk_relu(b)
                for rb in range(4):
                    for j, b in enumerate(r):
                        yield mk_sel1(rb, j, b)
                    yield mk_add(rb)
                return

            # last range: its work is the kernel tail — maximize overlap by
            # spreading accumulators across the freed pooling banks.
            def mk_mm_bank(b, bi, fc):
                def op():
                    if fc == 0:
                        t = ps_bank.tile([128, IN_F], f32, tag=f"bk{bi}",
                                         space="PSUM", name=f"pwL_{b}")
                        state[("pw", b)] = t
                    nc.tensor.matmul(state[("pw", b)][:, 0:HID],
                                     lhsT=state.pop((b, fc))[:],
                                     rhs=w1t[:, fc * HID:(fc + 1) * HID],
                                     start=(fc == 0), stop=(fc == 3),
                                     skip_group_check=True)
                return op

            def mk_relu_bank(b):
                def op():
                    nc.scalar.activation(h1t[:, b * HID:(b + 1) * HID],
                                         state.pop(("pw", b))[:, 0:HID],
                                         mybir.ActivationFunctionType.Relu)
                    p0tiles.pop(b)
                return op

            def mk_sel1_bank(rb, j, b):
                def op():
                    if j == 0:
                        state[("pl", rb)] = ps_bank.tile(
                            [128, IN_F], f32, tag=f"bk{rb}", space="PSUM",
                            name=f"plL_{rb}")
                    nc.tensor.matmul(
                        state[("pl", rb)][:, 0:HID],
                        lhsT=sel1_t[:, (b * 4 + rb) * 128:
                                    (b * 4 + rb) * 128 + 128],
                        rhs=h1t[:, b * HID:(b + 1) * HID],
                        start=(j == 0), stop=(j == len(r) - 1),
                        skip_group_check=True)
                return op

            def mk_add_bank(rb):
                def op():
                    pl = state.pop(("pl", rb))
                    if first_range:
                        nc.vector.tensor_copy(
                            pooled2[:, rb * HID:(rb + 1) * HID],
                            pl[:, 0:HID])
                    else:
                        nc.vector.tensor_tensor(
                            out=pooled2[:, rb * HID:(rb + 1) * HID],
                            in0=pooled2[:, rb * HID:(rb + 1) * HID],
                            in1=pl[:, 0:HID], op=mybir.AluOpType.add)
                return op

            def mk_w2s(rb, stage, hc=0):
                def op():
                    if stage == 0:
                        p2 = tpool.tile([128, HID], bf16, tag="p2",
                                        name=f"p2_{rb}")
                        nc.vector.tensor_copy(
                            p2[:], pooled2[:, rb * HID:(rb + 1) * HID])
                        state[("p2", rb)] = p2
                        state[("p2T", rb)] = tpool.tile(
                            [128, 2 * 128], bf16, tag="p2T", name=f"p2T_{rb}")
                    elif stage == 1:
                        p2 = state[("p2", rb)]
                        ptr = ps_tr.tile([128, 128], f32, tag="ptr",
                                         space="PSUM",
                                         name=f"ptrT_{rb}_{hc}")
                        nc.tensor.matmul(
                            ptr[:], lhsT=p2[:, hc * 128:(hc + 1) * 128],
                            rhs=ident[:], start=True, stop=True,
                            skip_group_check=True)
                        nc.vector.tensor_copy(
                            state[("p2T", rb)][:, hc * 128:(hc + 1) * 128],
                            ptr[:])
                    elif stage == 2:
                        if hc == 0:
                            state[("po", rb)] = ps_bank.tile(
                                [128, IN_F], f32, tag=f"bk{rb}",
                                space="PSUM", name=f"po_{rb}")
                        nc.tensor.matmul(
                            state[("po", rb)][0:NCLS, 0:128],
                            lhsT=w2t[:, hc * NCLS:(hc + 1) * NCLS],
                            rhs=state[("p2T", rb)][:, hc * 128:(hc + 1) * 128],
                            start=(hc == 0), stop=(hc == 1),
                            skip_group_check=True)
                    else:
                        osb = tpool.tile([NCLS, 128], f32, tag="osb",
                                         name=f"osb_{rb}")
                        nc.vector.tensor_copy(
                            osb[:], state.pop(("po", rb))[0:NCLS, 0:128])
                        nc.sync.dma_start(
                            outT.ap()[:, rb * 128:(rb + 1) * 128], osb[:])
                        state.pop(("p2", rb))
                        state.pop(("p2T", rb))
                return op

            for fc in range(4):
                for b in r:
                    yield mk_tr(b, fc)
            for fc in range(4):
                for bi, b in enumerate(r):
                    yield mk_mm_bank(b, bi, fc)
            for b in r:
                yield mk_relu_bank(b)
            for j, b in enumerate(r):
                for rb in range(4):
                    yield mk_sel1_bank(rb, j, b)
            for rb in range(4):
                yield mk_add_bank(rb)
                yield mk_w2s(rb, 0)
            for rb in range(4):
                yield mk_w2s(rb, 1, 0)
                yield mk_w2s(rb, 1, 1)
            for rb in range(4):
                yield mk_w2s(rb, 2, 0)
                yield mk_w2s(rb, 2, 1)
                yield mk_w2s(rb, 3)

        R = len(ranges)
        from collections import deque
        backlog = deque()

        def drain_backlog(k):
            for _ in range(k):
                if not backlog:
                    return
                backlog.popleft()()
        emit_state = {}

        for ri in range(R):
            for c in range(NCH):
                emit_pooling(ri, c, drain_backlog)
                if c == NCH - 1:
                    for b in ranges[ri]:
                        evict_piece(b)
                    b0, b1 = ranges[ri][0], ranges[ri][-1] + 1
                    nc.gpsimd.dma_start(
                        sel1_t[:, b0 * 512:b1 * 512],
                        sel1.ap()[:, b0 * 512:b1 * 512])
                    backlog.extend(back_ops(ri))
        while backlog:
            backlog.popleft()()

    nc.compile()
    return nc


def _get_nc(nb1=None, npb_key=None):
    if nb1 is None:
        # post-run introspection (test harness): return the built module
        return next(iter(_BUILT.values()))
    key = (nb1, npb_key)
    if key not in _BUILT:
        _BUILT[key] = build_nc(nb1, npb_key)
    return _BUILT[key]


def _repack(uniq, src0, nb1):
    """Assign unique slots to blocks s.t. every (block, chunk) has <= 128
    edges. Returns perm (position -> uniq rank) or None if repair failed."""
    nu = len(uniq)
    nsb = nb1
    # per-slot chunk histogram [nu, NCH]
    rows = src0[(uniq[:, None] * F + np.arange(F)[None, :])]
    ch = rows // CH
    hist = np.zeros((nu, NCH), np.int32)
    for k in range(F):
        np.add.at(hist, (np.arange(nu), ch[:, k]), 1)
    # initial: sequential assignment
    blk = np.arange(nu) // BLK
    cnt = np.zeros((nsb, NCH), np.int32)
    np.add.at(cnt, (blk, slice(None)), 0)
    for b in range(nsb):
        cnt[b] = hist[blk == b].sum(axis=0)
    for _ in range(400):
        viol = np.argwhere(cnt > BLK)
        if len(viol) == 0:
            return blk
        b, c = viol[0]
        members = np.where(blk == b)[0]
        s_out = members[np.argmax(hist[members, c])]
        b2 = int(np.argmin(cnt[:, c]))
        members2 = np.where(blk == b2)[0]
        if len(members2) == 0:
            blk[s_out] = b2
            cnt[b] -= hist[s_out]
            cnt[b2] += hist[s_out]
            continue
        s_in = members2[np.argmin(hist[members2, c])]
        blk[s_out], blk[s_in] = b2, b
        cnt[b] += hist[s_in] - hist[s_out]
        cnt[b2] += hist[s_out] - hist[s_in]
    return None


def _prep_edges(src0, src1):
    """Per-core slot tables and edge groupings.

    Returns (nb1, npb[nb1][13], per-core tuples)."""
    cores = []
    nu_max = 0
    for core in range(NC_N):
        s1 = src1[core * DST_PC * F:(core + 1) * DST_PC * F].astype(np.int64)
        uniq = np.unique(s1)
        nu_max = max(nu_max, len(uniq))
        cores.append((s1, uniq))
    nb1 = -(-nu_max // BLK)
    nslots = nb1 * BLK

    percore = []
    cnt = np.zeros((NC_N, nslots // BLK, NCH), np.int64)
    for core in range(NC_N):
        s1, uniq = cores[core]
        nu = len(uniq)
        blk = _repack(uniq, src0, nb1)
        slots = np.full(nslots, -1, np.int64)   # -1 = pad slot (no edges)
        if blk is not None:
            # position of slot rank i: block blk[i], next free lane
            fill = np.zeros(nslots // BLK, np.int64)
            pos_of_rank = np.zeros(nu, np.int64)
            for i in range(nu):
                b = blk[i]
                pos_of_rank[i] = b * BLK + fill[b]
                fill[b] += 1
            slots[pos_of_rank] = uniq
        else:
            slots[:nu] = uniq
            pos_of_rank = np.arange(nu)
        # edges only for real slots: u-major, k within
        real = np.where(slots >= 0)[0]
        src = src0[(slots[real][:, None] * F + np.arange(F)[None, :])
                   ].reshape(-1)
        u = np.repeat(real, F)
        chunk = src // CH
        sb = u // BLK
        key = sb * NCH + chunk
        order = np.argsort(key, kind="stable")
        src_s, u_s, key_s = src[order], u[order], key[order]
        seg = np.searchsorted(key_s, np.arange(nslots // BLK * NCH + 1))
        cnt[core] = (seg[1:] - seg[:-1]).reshape(nslots // BLK, NCH)
        percore.append((s1, uniq, slots, src_s, u_s, seg, pos_of_rank))
    npb = np.maximum(1, -(-cnt.max(axis=0) // 128))
    return nb1, npb, percore


def _prep_core_tables(nb1, npb, core_data):
    """Build gidx (wrapped int16), slotw, sel1 for one core."""
    s1, uniq, slots, src_s, u_s, seg, pos_of_rank = core_data
    ranges = _ranges(nb1)
    tot_pos = 0
    cols = []
    for ri, r in enumerate(ranges):
        for c in range(NCH):
            for sb in r:
                tot_pos += npb[sb][c] * 128
    gidx = np.zeros((16, tot_pos // 16), np.int16)
    ncol = int(npb.sum())
    slotw = np.full((128, ncol), -1.0, np.float32)  # cast to bf16 at return

    pos0 = 0
    col = 0
    for ri, r in enumerate(ranges):
        for c in range(NCH):
            for sb in r:
                g = sb * NCH + c
                lo, hi = seg[g], seg[g + 1]
                n = hi - lo
                cap = npb[sb][c] * 128
                rel = np.zeros(cap, np.int64)
                rel[:n] = src_s[lo:hi] - c * CH
                w = np.full(cap, -1.0, np.float32)
                w[:n] = (u_s[lo:hi] % BLK).astype(np.float32)
                for piece in range(npb[sb][c]):
                    pw = w[piece * 128:(piece + 1) * 128]
                    slotw[:, col] = pw
                    col += 1
                # wrap16 into the gather's column range (positions pos0..)
                i = np.arange(cap)
                gidx[(pos0 + i) % 16, (pos0 + i) // 16] = rel.astype(np.int16)
                pos0 += cap
    gidx_full = np.tile(gidx, (8, 1))

    # layer-1 SEL (v4's layout), slot positions via the repacking
    nslots = nb1 * BLK
    e_slot = pos_of_rank[np.searchsorted(uniq, s1)]
    e_dst = np.arange(DST_PC * F) // F
    S = np.zeros((nslots, DST_PC), np.float32)
    np.add.at(S, (e_slot, e_dst), 1.0)
    S4 = S.reshape(nb1, BLK, 4, 128).transpose(1, 0, 2, 3).reshape(
        BLK, nb1 * 4 * 128)
    return (np.ascontiguousarray(gidx_full),
            np.ascontiguousarray(slotw).astype(ml_dtypes.bfloat16),
            np.ascontiguousarray(S4).astype(ml_dtypes.float8_e4m3fn))


def _run(inputs, trace=False, trace_kwargs=None):
    from concourse.bass_utils import run_bass_kernel_spmd

    featb = np.ascontiguousarray(
        np.asarray(inputs["features"], dtype=np.float32)
    ).astype(ml_dtypes.bfloat16)
    W1 = np.ascontiguousarray(inputs["W1"], dtype=np.float32) / np.float32(F)
    W2 = np.ascontiguousarray(inputs["W2"], dtype=np.float32) / np.float32(F)
    b1 = np.ascontiguousarray(inputs["b1"], dtype=np.float32)
    b2 = np.ascontiguousarray(inputs["b2"], dtype=np.float32)
    src0 = np.asarray(inputs["src0"]).astype(np.int64)
    src1 = np.asarray(inputs["src1"]).astype(np.int64)
    assert np.abs(b1).max() == 0.0, "nonzero b1 handled by numpy fallback"

    # w1t[p, fc*256+h] = W1[fc*128+p, h]; w2t[p, hc*64+n] = W2[hc*128+p, n]
    w1t = np.ascontiguousarray(
        W1.reshape(4, 128, HID).transpose(1, 0, 2).reshape(128, 4 * HID)
    ).astype(ml_dtypes.bfloat16)
    w2t = np.ascontiguousarray(
        W2.reshape(2, 128, NCLS).transpose(1, 0, 2).reshape(128, 2 * NCLS)
    ).astype(ml_dtypes.bfloat16)
    ident = np.eye(128, dtype=np.float32).astype(ml_dtypes.bfloat16)
    iota = np.ascontiguousarray(np.broadcast_to(
        np.arange(128, dtype=np.float32)[None, :], (128, 128))
    ).astype(ml_dtypes.bfloat16)

    nb1, npb, percore = _prep_edges(src0, src1)
    npb_key = tuple(int(x) for x in npb.reshape(-1))

    in_maps = []
    for core in range(NC_N):
        gidx, slotw, S4 = _prep_core_tables(nb1, npb, percore[core])
        in_maps.append({"feat": featb, "w1": w1t, "w2": w2t,
                        "ident": ident, "iota": iota,
                        "gidx": gidx, "slw": slotw, "sel1": S4})
    nc = _get_nc(nb1, npb_key)
    kw = {}
    if trace:
        kw = {"trace": True, "trace_kwargs": trace_kwargs or {}}
    res = run_bass_kernel_spmd(nc, in_maps, list(range(NC_N)), **kw)
    full = np.concatenate(
        [np.ascontiguousarray(res.results[c]["outT"].T) for c in range(NC_N)],
        axis=0)
    full = full + b2[None, :]
    return full, res


def kernel(features, W1, b1, W2, b2, src0, dst0, src1, dst1):
    ins = dict(features=features, W1=W1, b1=b1, W2=W2, b2=b2,
               src0=src0, dst0=dst0, src1=src1, dst1=dst1)
    d0 = np.asarray(dst0); d1 = np.asarray(dst1)
    fixed = (d0 == np.arange(N1 * F) // F).all() and \
            (d1 == np.arange(N2 * F) // F).all() and \
            np.abs(np.asarray(b1)).max() == 0.0
    if not fixed:
        f = np.asarray(features, dtype=np.float64)
        m = f[np.asarray(src0)]
        s = np.zeros((N1, IN_F)); np.add.at(s, d0, m)
        deg = np.bincount(d0, minlength=N1).clip(1)
        h = np.maximum(s / deg[:, None] @ np.asarray(W1) + np.asarray(b1), 0)
        m = h[np.asarray(src1)]
        s = np.zeros((N2, HID)); np.add.at(s, d1, m)
        deg = np.bincount(d1, minlength=N2).clip(1)
        return ((s / deg[:, None]) @ np.asarray(W2) + np.asarray(b2)
                ).astype(np.float32)
    out, _ = _run(ins)
    return out


if __name__ == "__main__":
    import reference  # only available next to the dev checkout
    import numpy as _np
    ins = {k: _np.asarray(v) for k, v in reference.setup_inputs().items()}
    out, _ = _run(ins)
    from concourse.timeline_sim import TimelineSim
    ts = TimelineSim(_get_nc(), trace=False)
    ts.simulate()
    print("TimelineSim:", int(ts.time), "ns")
